# revision 22
# baseline (speedup 1.0000x reference)
"""Additive (Bahdanau) attention on 8 TRN2 NeuronCores (raw Bass).

Reference math (B=4, Tq=256, Tk=512, Dq=Dv=512, U=256):
    q = query @ W1                      [B,Tq,U]
    k = value @ W2                      [B,Tk,U]
    scores[b,t,s] = sum_u scale[u] * tanh(q[b,t,u] + k[b,s,u])
    attn = softmax(scores, axis=-1)     [B,Tq,Tk]
    context = attn @ value              [B,Tq,Dv]
    returns (context, attn)

Sharding: (b, tq-half) -> 8 cores, 128 query rows each; Tk stays local so
there are no collectives.  Per-core dataflow keeps U on partitions for the
big [t,s,u] stage:
    DVE:  X[u, (t,s)] = k[u,s] + q[u,t]   (tensor_scalar add, bf16 4x mode)
    ACT:  T = tanh(X)                     (one big activation per t-block)
    PE :  scoresT[s,t] = sum_u scale[u] T[u,s]   (per-t matvecs, T stationary)
    ACT:  E = exp(scoresT)                (softmax without max: |scores|<~13)
    PE :  sums[t] = E.T @ 1, ctx_raw = E.T @ value, attnT = transpose(E)
    DVE:  r = 1/sums; outputs scaled by r (per-partition scalar)

Engineering notes:
  - this walrus allows only ONE attached sync-wait per instruction, so all
    waits are standalone wait_ge instructions per engine (raw bass).
  - per-input-DMA semaphores: HWDGE completions are NOT FIFO across DMAs.
  - the host passes PRE-TRANSPOSED bf16 operands (queryT, valueT, bf16
    weights/value) - no on-chip input transposes and half the DMA bytes.
    critical loads are spread over four DMA paths (sync+scalar HWDGE,
    gpsimd+vector SWDGE) so the k projection starts ~10us in.
  - the DVE's scalar operand (tensor_scalar/activation bias) is prefetched
    by the sequencer BEFORE the previous op's writes drain, so a value
    produced by the immediately-preceding DVE op needs a drain or an
    intervening op before it is consumed as a scalar.
  - softmax/context/attn run in four UNEVEN t-groups (40/40/32/16 rows):
    groups 0-2 are processed under the tanh stream of later t-blocks and
    only the tiny 16-row group 3 remains in the tail.
"""

from contextlib import ExitStack

import numpy as np

import concourse.bass as bass
import concourse.mybir as mybir
from concourse.bass_utils import run_bass_kernel_spmd

F32 = mybir.dt.float32
BF16 = mybir.dt.bfloat16
AF = mybir.ActivationFunctionType

N_CORES = 8
B, TQ, TK, DQ, DV, U = 4, 256, 512, 512, 512, 256
T_ROWS = 128          # query rows per core
UC = U // 128         # u chunks (2)
DC = DQ // 128        # d chunks (4)
SC = TK // 128        # s chunks (4)
TB = 8                # t-block size for the tanh pipeline
NTB = T_ROWS // TB    # 16
XFREE = UC * TB * TK  # 8192 free elems per X/T buffer

# phase-2 groups: (t0, n_rows), score-tile base col, attnT base col,
# slots: exp after tanh tb / pe after mv tb / recip after adds tb /
#        muls after adds tb  (None = after the loop)
GROUPS = [
    dict(t0=0, n=40, col=0, att=1024, exp=5, pe=5, rc=9, mul=10),
    dict(t0=40, n=40, col=512, att=1536, exp=10, pe=10, rc=14, mul=15),
    dict(t0=80, n=32, col=160, att=1280, exp=14, pe=14, rc=None, mul=None),
    dict(t0=112, n=16, col=672, att=1792, exp=None, pe=None, rc=None, mul=None),
]


def grp_of(t):
    for gi, g in enumerate(GROUPS):
        if g["t0"] <= t < g["t0"] + g["n"]:
            return gi, g
    raise AssertionError


def build_bass() -> bass.Bass:
    nc = bass.Bass()
    # all inputs pre-packed host-side into SBUF layout [128, free] so each
    # DMA moves large contiguous per-partition runs
    qt_ext = nc.declare_dram_parameter("queryT", [128, DC * 128], BF16, isOutput=False)
    vt_ext = nc.declare_dram_parameter("valueT", [128, DC * TK], BF16, isOutput=False)
    vb_ext = nc.declare_dram_parameter("valuebf", [128, SC * DV], BF16, isOutput=False)
    w1_ext = nc.declare_dram_parameter("W1b", [128, DC * U], BF16, isOutput=False)
    w2_ext = nc.declare_dram_parameter("W2b", [128, DC * U], BF16, isOutput=False)
    scl_ext = nc.declare_dram_parameter("scaleb", [128, UC], BF16, isOutput=False)
    idb_ext = nc.declare_dram_parameter("identb", [128, 128], BF16, isOutput=False)
    ctx_ext = nc.declare_dram_parameter("context", [T_ROWS, DV], F32, isOutput=True)
    attn_ext = nc.declare_dram_parameter("attn", [T_ROWS, TK], F32, isOutput=True)

    es = ExitStack()
    with es:
        _n = [0]

        def sb(shape, dt):
            _n[0] += 1
            return es.enter_context(nc.sbuf_tensor(f"sb{_n[0]}", shape, dt))

        # ---- SBUF ----
        vTb = sb([128, DC * TK], BF16)         # [d_p, (dc, s)]
        qTb = sb([128, DC * 128], BF16)        # [d_p, (dc, t)]
        w1b = sb([128, DC * U], BF16)          # [d_p, (dc, u)]
        w2b = sb([128, DC * U], BF16)
        v_bf = sb([128, SC * DV], BF16)        # [s_p, (sc, d)]
        scale_bf = sb([128, UC], BF16)
        ones_bf = sb([128, 1], BF16)
        ident_bf = sb([128, 128], BF16)
        q_f = sb([128, UC * 128], F32)         # [u_p, (uc, t)]
        k_bf = sb([128, UC * TK], BF16)        # [u_p, (uc, s)]
        X0 = sb([128, XFREE], BF16)
        X1 = sb([128, XFREE], BF16)
        X2 = sb([128, XFREE], BF16)
        T0 = sb([128, XFREE], BF16)
        T1 = sb([128, XFREE], BF16)
        E_G = [sb([128, SC * g["n"]], BF16) for g in GROUPS]  # [s_p, (sc, t)]
        r_G = [sb([128, 1], F32) for _ in GROUPS]
        ctx_G = [sb([128, DV], F32) for _ in GROUPS]          # rows 0:n used
        attn_G = [sb([128, TK], F32) for _ in GROUPS]
        Xs, Ts = [X0, X1, X2], [T0, T1]

        # tanh segments: (tb, lo_tl, hi_tl); tb0 and tb15 are split in half
        TANH_SEGS = (
            [(0, 0, 4), (0, 4, 8)]
            + [(tb, 0, 8) for tb in range(1, 15)]
            + [(15, 0, 4), (15, 4, 8)]
        )
        SEG_ADD_WAIT = [1, 2] + [tb + 2 for tb in range(1, 15)] + [17, 17]

        def mv_tanh_thresh(tb, tl):
            if tb == 0:
                return 1 if tl < 4 else 2
            if tb == 15:
                return 17 if tl < 4 else 18
            return tb + 2

        # ---- PSUM ----
        ringA = es.enter_context(nc.psum_tensor("ringA", [128, 2048], F32))
        ringB = es.enter_context(nc.psum_tensor("ringB", [128, 2048], F32))
        k_ps = [ringB[:, 1024:1536], ringB[:, 1536:2048]]
        q_ps = [ringB[:, 0:128], ringB[:, 512:640]]
        # sums/ctx banks alternate b6/b7 and b4/b5 per group
        sums_G = [
            ringB[0 : GROUPS[i]["n"], 1024 + (i % 2) * -1024 :][:, 0:1]
            for i in range(4)
        ]
        sums_G = [
            ringB[0 : GROUPS[0]["n"], 1024:1025],
            ringB[0 : GROUPS[1]["n"], 0:1],
            ringB[0 : GROUPS[2]["n"], 1024:1025],
            ringB[0 : GROUPS[3]["n"], 0:1],
        ]
        ctxp_G = [
            ringB[0 : GROUPS[0]["n"], 1536:2048],
            ringB[0 : GROUPS[1]["n"], 512:1024],
            ringB[0 : GROUPS[2]["n"], 1536:2048],
            ringB[0 : GROUPS[3]["n"], 512:1024],
        ]

        def att_tile(i, sc):
            b = GROUPS[i]["att"]
            return ringA[:, b + sc * 64 : b + (sc + 1) * 64].bitcast(BF16)

        def att_all(i):
            b = GROUPS[i]["att"]
            return ringA[:, b : b + 256].bitcast(BF16)

        sem = lambda name: es.enter_context(nc.semaphore(name))
        s_vtA = sem("s_vtA")   # vT cols 0:1024 (dc 0,1)
        s_vtB = sem("s_vtB")   # vT cols 1024:2048 (dc 2,3)
        s_qt = sem("s_qt")
        s_w1 = sem("s_w1")
        s_w2 = sem("s_w2")
        s_scl = sem("s_scl")
        s_idb = sem("s_idb")
        s_vbf = sem("s_vbf")
        s_proj = sem("s_proj")    # k0,k1,q0,q1
        s_evac = sem("s_evac")    # q_f, k_bf
        s_add = sem("s_add")      # 17 (tb0 split)
        s_tanh = sem("s_tanh")    # 18 (tb0/tb15 split)
        s_mv = sem("s_mv")        # 16
        s_exp = sem("s_exp")      # 4
        s_sums = sem("s_sums")    # 4
        s_ctxs = sem("s_ctxs")    # 4
        s_att = sem("s_att")      # 16
        s_o = [sem(f"s_o{i}") for i in range(4)]  # ctx=1, attn=2
        s_dout = sem("s_dout")    # 128

        def phase2_pe(tensor, i):
            g = GROUPS[i]
            n = g["n"]
            E = E_G[i]
            tensor.wait_ge(s_exp, i + 1)
            if i == 0:
                tensor.wait_ge(s_vbf, 16)
                tensor.wait_ge(s_idb, 16)
            if i >= 2:
                tensor.wait_ge(s_o[i - 2], 1)  # sums/ctx bank readers done
            for sc in range(SC):
                ins = tensor.matmul(
                    out=sums_G[i],
                    lhsT=E[:, sc * n : (sc + 1) * n],
                    rhs=ones_bf[:, 0:1],
                    start=(sc == 0),
                    stop=(sc == SC - 1),
                )
            ins.then_inc(s_sums, 1)
            for sc in range(SC):
                ins = tensor.matmul(
                    out=ctxp_G[i],
                    lhsT=E[:, sc * n : (sc + 1) * n],
                    rhs=v_bf[:, sc * DV : (sc + 1) * DV],
                    start=(sc == 0),
                    stop=(sc == SC - 1),
                )
            ins.then_inc(s_ctxs, 1)
            if i >= 2:
                tensor.wait_ge(s_o[i - 2], 2)  # attnT bank readers done
            for sc in range(SC):
                tensor.transpose(
                    out=att_tile(i, sc)[0:n, :],
                    in_=E[:, sc * n : (sc + 1) * n],
                    identity=ident_bf[:, :],
                ).then_inc(s_att, 1)

        def rc_dve(vector, i):
            # reciprocal in its own slot + drain: r is consumed as a scalar
            # operand later and scalar reads bypass the DVE pipe
            n = GROUPS[i]["n"]
            vector.wait_ge(s_sums, i + 1)
            vector.reciprocal(out=r_G[i][0:n, :], in_=sums_G[i])
            vector.drain()

        def mul_dve(vector, i):
            n = GROUPS[i]["n"]
            vector.wait_ge(s_ctxs, i + 1)
            vector.tensor_scalar_mul(
                out=ctx_G[i][0:n, :], in0=ctxp_G[i], scalar1=r_G[i][0:n, 0:1]
            ).then_inc(s_o[i], 1)
            vector.wait_ge(s_att, 4 * i + 4)
            vector.tensor_scalar_mul(
                out=attn_G[i][0:n, :],
                in0=att_all(i)[0:n, :],
                scalar1=r_G[i][0:n, 0:1],
            ).then_inc(s_o[i], 1)

        with nc.Block() as block:

            @block.sync
            def _(sync):
                sync.dma_start(
                    out=vTb[:, 0 : 2 * TK], in_=vt_ext[:, 0 : 2 * TK]
                ).then_inc(s_vtA, 16)
                sync.dma_start(out=qTb[:, :], in_=qt_ext[:, :]).then_inc(s_qt, 16)
                sync.dma_start(out=w1b[:, :], in_=w1_ext[:, :]).then_inc(s_w1, 16)
                for i in range(4):
                    g = GROUPS[i]
                    sync.wait_ge(s_o[i], 1)
                    sync.dma_start(
                        out=ctx_ext[g["t0"] : g["t0"] + g["n"], :],
                        in_=ctx_G[i][0 : g["n"], :],
                    ).then_inc(s_dout, 16)
                    sync.wait_ge(s_o[i], 2)
                    sync.dma_start(
                        out=attn_ext[g["t0"] : g["t0"] + g["n"], :],
                        in_=attn_G[i][0 : g["n"], :],
                    ).then_inc(s_dout, 16)
                sync.wait_ge(s_dout, 128)

            @block.scalar
            def _(scalar):
                scalar.dma_start(out=w2b[:, :], in_=w2_ext[:, :]).then_inc(
                    s_w2, 16
                )
                scalar.dma_start(
                    out=vTb[:, 2 * TK : 4 * TK], in_=vt_ext[:, 2 * TK : 4 * TK]
                ).then_inc(s_vtB, 16)
                # phase 1: tanh stream with group exps woven in
                prev_tb = -1
                exp_at = {g["exp"]: i for i, g in enumerate(GROUPS) if g["exp"]}
                for k, (tb, lo, hi) in enumerate(TANH_SEGS):
                    scalar.wait_ge(s_add, SEG_ADD_WAIT[k])
                    if tb != prev_tb and tb >= 2:
                        scalar.wait_ge(s_mv, tb - 1)
                    prev_tb = tb
                    scalar.activation(
                        out=Ts[tb % 2][:, lo * UC * TK : hi * UC * TK],
                        in_=Xs[tb % 3][:, lo * UC * TK : hi * UC * TK],
                        func=AF.Tanh,
                    ).then_inc(s_tanh, 1)
                    if hi == 8 and tb in exp_at:
                        i = exp_at[tb]
                        g = GROUPS[i]
                        scalar.wait_ge(s_mv, tb)
                        scalar.activation(
                            out=E_G[i][:, :],
                            in_=ringA[:, g["col"] : g["col"] + SC * g["n"]],
                            func=AF.Exp,
                        ).then_inc(s_exp, 1)
                scalar.wait_ge(s_mv, NTB)
                g = GROUPS[3]
                scalar.activation(
                    out=E_G[3][:, :],
                    in_=ringA[:, g["col"] : g["col"] + SC * g["n"]],
                    func=AF.Exp,
                ).then_inc(s_exp, 1)

            @block.gpsimd
            def _(gpsimd):
                gpsimd.dma_start(out=scale_bf[:, :], in_=scl_ext[:, :]).then_inc(
                    s_scl, 16
                )
                gpsimd.dma_start(out=ident_bf[:, :], in_=idb_ext[:, :]).then_inc(
                    s_idb, 16
                )
                gpsimd.dma_start(out=v_bf[:, :], in_=vb_ext[:, :]).then_inc(
                    s_vbf, 16
                )

            @block.vector
            def _(vector):
                vector.memset(ones_bf[:, :], 1.0)
                # evacuations: q first, then k (the k copy separates the q_f
                # write from the adds' scalar prefetch)
                rB3 = ringB[:, :].rearrange("p (b x) -> p b x", b=4)
                vector.wait_ge(s_proj, 4)
                vector.tensor_copy(out=q_f[:, :], in_=rB3[:, 0:2, 0:128]).then_inc(
                    s_evac, 1
                )
                vector.tensor_copy(out=k_bf[:, :], in_=ringB[:, 1024:2048]).then_inc(
                    s_evac, 1
                )
                # phase 1 adds with group epilogue pieces woven in
                rc_at = {g["rc"]: i for i, g in enumerate(GROUPS) if g["rc"]}
                mul_at = {g["mul"]: i for i, g in enumerate(GROUPS) if g["mul"]}
                for tb in range(NTB):
                    buf = Xs[tb % 3]
                    if tb >= 3:
                        vector.wait_ge(s_tanh, tb - 1)
                    for tl in range(TB):
                        t = tb * TB + tl
                        for uc in range(UC):
                            ins = vector.tensor_scalar_add(
                                out=buf[
                                    :, (tl * UC + uc) * TK : (tl * UC + uc + 1) * TK
                                ],
                                in0=k_bf[:, uc * TK : (uc + 1) * TK],
                                scalar1=q_f[:, uc * 128 + t : uc * 128 + t + 1],
                            )
                        if tb == 0 and tl == 3:
                            ins.then_inc(s_add, 1)
                    ins.then_inc(s_add, 1)
                    if tb in rc_at:
                        rc_dve(vector, rc_at[tb])
                    if tb in mul_at:
                        mul_dve(vector, mul_at[tb])
                # remaining group epilogues
                rc_dve(vector, 2)
                mul_dve(vector, 2)
                rc_dve(vector, 3)
                mul_dve(vector, 3)

            @block.tensor
            def _(tensor):
                # k projection - starts as soon as vT chunks + W2 land
                tensor.wait_ge(s_w2, 16)
                for uc in range(UC):
                    for dc in range(DC):
                        if uc == 0 and dc == 0:
                            tensor.wait_ge(s_vtA, 16)
                        if uc == 0 and dc == 2:
                            tensor.wait_ge(s_vtB, 16)
                        ins = tensor.matmul(
                            out=k_ps[uc],
                            lhsT=w2b[:, dc * U + uc * 128 : dc * U + uc * 128 + 128],
                            rhs=vTb[:, dc * TK : (dc + 1) * TK],
                            start=(dc == 0),
                            stop=(dc == DC - 1),
                        )
                    ins.then_inc(s_proj, 1)
                tensor.wait_ge(s_qt, 16)
                tensor.wait_ge(s_w1, 16)
                for uc in range(UC):
                    for dc in range(DC):
                        ins = tensor.matmul(
                            out=q_ps[uc],
                            lhsT=w1b[:, dc * U + uc * 128 : dc * U + uc * 128 + 128],
                            rhs=qTb[:, dc * 128 : (dc + 1) * 128],
                            start=(dc == 0),
                            stop=(dc == DC - 1),
                        )
                    ins.then_inc(s_proj, 1)
                tensor.wait_ge(s_scl, 16)
                # phase 1: score matvecs; group phase-2 woven in
                pe_at = {g["pe"]: i for i, g in enumerate(GROUPS) if g["pe"]}
                for tb in range(NTB):
                    tensor.wait_ge(s_tanh, mv_tanh_thresh(tb, 0))
                    Tt = Ts[tb % 2]
                    for tl in range(TB):
                        if tb in (0, 15) and tl == 4:
                            tensor.wait_ge(s_tanh, mv_tanh_thresh(tb, 4))
                        t = tb * TB + tl
                        gi, g = grp_of(t)
                        col = g["col"] + (t - g["t0"])
                        for sc in range(SC):
                            for uc in range(UC):
                                base = (tl * UC + uc) * TK + sc * 128
                                ins = tensor.matmul(
                                    out=ringA[:, col + sc * g["n"] :][:, 0:1],
                                    lhsT=Tt[:, base : base + 128],
                                    rhs=scale_bf[:, uc : uc + 1],
                                    start=(uc == 0),
                                    stop=(uc == UC - 1),
                                )
                    ins.then_inc(s_mv, 1)
                    if tb in pe_at:
                        phase2_pe(tensor, pe_at[tb])
                phase2_pe(tensor, 3)

    return nc


_NC = None


def _get_nc() -> bass.Bass:
    global _NC
    if _NC is None:
        _NC = build_bass()
    return _NC


_CONST = None


def make_in_maps(query, value, W1, W2, scale):
    global _CONST
    import ml_dtypes

    bf = ml_dtypes.bfloat16
    if _CONST is None:
        _CONST = {"identb": np.eye(128).astype(bf)}
    query = np.asarray(query, dtype=np.float32)
    value = np.asarray(value, dtype=np.float32)
    W1 = np.asarray(W1, np.float32)
    W2 = np.asarray(W2, np.float32)
    scaleb = np.ascontiguousarray(
        np.asarray(scale, np.float32).reshape(UC, 128).T.astype(bf)
    )
    in_maps = []
    for c in range(N_CORES):
        b, th = c // 2, c % 2
        qloc = query[b, th * T_ROWS : (th + 1) * T_ROWS, :]
        vloc = value[b]
        # pack [D, X] operands into SBUF layout [128, (chunk, x)]
        pk = lambda a: np.ascontiguousarray(
            a.reshape(4, 128, a.shape[1]).transpose(1, 0, 2).reshape(128, -1)
        )
        in_maps.append(
            {
                "queryT": pk(qloc.T.astype(bf)),
                "valueT": pk(vloc.T.astype(bf)),
                "valuebf": pk(vloc.astype(bf)),
                "W1b": pk(W1.astype(bf)),
                "W2b": pk(W2.astype(bf)),
                "scaleb": scaleb,
                "identb": _CONST["identb"],
            }
        )
    return in_maps


def assemble(results):
    context = np.empty((B, TQ, DV), dtype=np.float32)
    attn = np.empty((B, TQ, TK), dtype=np.float32)
    for c in range(N_CORES):
        b, th = c // 2, c % 2
        context[b, th * T_ROWS : (th + 1) * T_ROWS, :] = results[c]["context"]
        attn[b, th * T_ROWS : (th + 1) * T_ROWS, :] = results[c]["attn"]
    return context, attn


def kernel(query, value, W1, W2, scale):
    nc = _get_nc()
    in_maps = make_in_maps(query, value, W1, W2, scale)
    res = run_bass_kernel_spmd(nc, in_maps, core_ids=list(range(N_CORES)))
    return assemble(res.results)
